# revision 29
# baseline (speedup 1.0000x reference)
"""Trainium2 Bass kernel for channel attention (XCA-style) over 8 NeuronCores.

Module: q/k/v = depthwise3x3(1x1conv(x)); L2-normalize q,k over spatial;
channel attention per head (softmax over d=64 channel dim); 1x1 proj.

Sharding: data-parallel over batch B=8 -> 1 batch item per core, no
collectives.  All shapes hardcoded; weights pre-transposed on host.

v2 design (per core, per 128-channel chunk):
- 1x1 convs: bf16 PE matmuls into PSUM, ACT-copied into a zero-bordered
  [66, 68] padded SBUF tile (stride 68 keeps window reads 4B-aligned).
- depthwise 3x3 split by tap across engines: PE takes the dx=1 column
  (3 taps) as accumulating diag-matmuls (diags precomputed on host),
  ACT drains the partial to SBUF, DVE adds the 6 dx={0,2} taps as
  TS(4x mode) + TT-add(2x mode) pairs.  Some chunks run all 9 taps on
  PE (DW_MODE knob) for engine balance.
- L2 norm: ACT Square+accum_out (scratch over the dead pad tile),
  Newton-refined rsqrt; q's 1/|q| folds into the softmax exp scale,
  k's into a DVE 4x row-scale.
- q/k transposed to [n, c] via DMA xbar transpose (free on compute).
- softmax 1/sum folds into a row-scale of exp(attn); attn@v and the
  output projection fuse: A_g = attn_g.T @ wp_g (tiny matmul), then
  y_m = sum_g A_g[:,m].T @ vdw_g -- the separate attn@v pass and its
  PSUM drain disappear.
"""

import os
import sys

import numpy as np

for _p in ("/opt/trn_rl_repo", "/root/.axon_site/_ro/trn_rl_repo"):
    if os.path.isdir(_p) and _p not in sys.path:
        sys.path.insert(0, _p)

import ml_dtypes

B, C, HH, WW = 8, 512, 64, 64
HEADS, D = 8, 64
HW = HH * WW          # 4096
G = C // 128          # 4 channel chunks of 128
NBK = 512             # matmul N (one PSUM bank of fp32)
NB = HW // NBK        # 8
PR = 66               # pad row stride (elems); data at [1+y, 1+x]
PY = HH + 2           # 66 pad rows
EPS = 1e-12

# PE tap order: center column (dx=1) first so start=True covers full banks
TAPORD = [4, 1, 7, 3, 5, 0, 2, 6, 8]
# per (t,g) depthwise split:
#   "col" = taps {4,1,7} on PE + 6 on DVE
#   "all" = all 9 taps on PE (q/k tail chunks: transpose gates on ACT,
#           not on DVE's tap backlog)
#   "dve" = all 9 taps on DVE (v chunks: vdw only needed at proj, so
#           their taps can trail off the critical path)
DW_MODE = {("q", 2): "all", ("k", 2): "all", ("q", 3): "all",
           ("k", 3): "all", ("v", 3): "all",
           ("v", 0): "dve", ("v", 1): "dve", ("v", 2): "dve"}
N_PE = {"col": 3, "all": 9, "dve": 0}
# chunks whose pw PSUM->pad copies run on DVE (fills DVE's startup idle)
PW_COPY_DVE = {("q", 0)}
# chunks whose dw-partial PSUM->acc copies run on DVE
DW_COPY_DVE = {("q", 0)}
# load x in quarters (vs halves) so the first pw bank-pair starts earlier
X_QUARTERS = True
# (t,g) -> taps offloaded to GPSIMD STT (runs after that chunk's DVE taps)
GP_TAPS = {}
# route k transposes through the Activation HWDGE queue (parallel with
# q transposes on the SP queue) so attn pairs unblock ~5us earlier
K_TRANSPOSE_ON_ACT = False
# main schedule (program order drives Tile's priorities)
ORDER_CFG = [("q", 0), ("k", 0), ("q", 1), ("k", 1), ("v", 0), ("A", 0),
             ("q", 2), ("k", 2), ("v", 1), ("A", 1), ("v", 2), ("q", 3),
             ("k", 3), ("A", 2), ("v", 3), ("A", 3)]


def _mode(t, g):
    return DW_MODE.get((t, g), "col")


_CACHE = {}


def _build():
    """Build the single-core Bass program (SPMD: same program, per-core data)."""
    from contextlib import ExitStack

    import concourse.bass as bass
    import concourse.tile as tile
    from concourse import bacc, mybir

    f32 = mybir.dt.float32
    bf16 = mybir.dt.bfloat16
    AO = mybir.AluOpType
    AF = mybir.ActivationFunctionType

    nc = bacc.Bacc()

    x_ext = nc.declare_dram_parameter("x", [C, HW], bf16, isOutput=False)
    w_ext = {
        t: nc.declare_dram_parameter(f"w{t}", [C, C], bf16, isOutput=False)
        for t in "qkv"
    }
    wp_ext = nc.declare_dram_parameter("wp", [C, C], bf16, isOutput=False)
    # host-precomputed diag(dw tap) blocks, TAPORD order per chunk
    dg_ext = {
        t: nc.declare_dram_parameter(f"dg{t}", [128, G * 9 * 128], bf16,
                                     isOutput=False)
        for t in "qkv"
    }
    dw_ext = {
        t: nc.declare_dram_parameter(f"dw{t}", [C, 9], f32, isOutput=False)
        for t in "qkv"
    }
    tsc_ext = nc.declare_dram_parameter("tsc", [C, 1], f32, isOutput=False)
    out_ext = nc.declare_dram_parameter("out", [C, HW], f32, isOutput=True)

    with ExitStack() as ctx:
        tc = ctx.enter_context(tile.TileContext(nc))
        sb = ctx.enter_context(tc.tile_pool(name="sb", bufs=1))
        ps = ctx.enter_context(tc.tile_pool(name="ps", bufs=1, space="PSUM"))

        # ---- persistent loads -------------------------------------------
        x_sb = [None] * G
        w_sb = {t: [None] * G for t in "qkv"}
        # leading slices of x land early so pw bank-pair 0 starts ~4us in
        XCUT = HW // 4 if X_QUARTERS else HW // 2
        for k in range(G):
            wt = sb.tile([128, C], bf16, name=f"wq{k}", tag=f"wq{k}")
            nc.sync.dma_start(out=wt, in_=w_ext["q"][k * 128:(k + 1) * 128, :])
            w_sb["q"][k] = wt
            xg = sb.tile([128, HW], bf16, name=f"x{k}", tag=f"x{k}")
            nc.sync.dma_start(out=xg[:, 0:XCUT],
                              in_=x_ext[k * 128:(k + 1) * 128, 0:XCUT])
            x_sb[k] = xg
        if X_QUARTERS:
            for k in range(G):
                nc.sync.dma_start(out=x_sb[k][:, XCUT:HW // 2],
                                  in_=x_ext[k * 128:(k + 1) * 128,
                                            XCUT:HW // 2])
        for k in range(G):
            nc.sync.dma_start(out=x_sb[k][:, HW // 2:HW],
                              in_=x_ext[k * 128:(k + 1) * 128, HW // 2:HW])

        dg_sb = {}
        dw_sb = {t: [None] * G for t in "qkv"}
        tsc_sb = []

        def load_dg_dw(t):
            for g in range(G):
                n = N_PE[_mode(t, g)]
                if n:
                    d = sb.tile([128, n * 128], bf16, name=f"dg{t}{g}",
                                tag=f"dg{t}{g}")
                    nc.sync.dma_start(
                        out=d,
                        in_=dg_ext[t][:, g * 9 * 128:(g * 9 + n) * 128])
                    dg_sb[(t, g)] = d
                dd = sb.tile([128, 9], f32, name=f"dw{t}{g}", tag=f"dw{t}{g}")
                nc.sync.dma_start(out=dd,
                                  in_=dw_ext[t][g * 128:(g + 1) * 128, :])
                dw_sb[t][g] = dd

        load_dg_dw("q")
        for g in range(G):
            tg = sb.tile([128, 1], f32, name=f"tsc{g}", tag=f"tsc{g}")
            nc.sync.dma_start(out=tg, in_=tsc_ext[g * 128:(g + 1) * 128, :])
            tsc_sb.append(tg)
        for t in "kv":
            for k in range(G):
                wt = sb.tile([128, C], bf16, name=f"w{t}{k}", tag=f"w{t}{k}")
                nc.sync.dma_start(out=wt, in_=w_ext[t][k * 128:(k + 1) * 128, :])
                w_sb[t][k] = wt
            load_dg_dw(t)
        wp_sb = []
        for k in range(G):
            wpt = sb.tile([128, C], bf16, name=f"wp{k}", tag=f"wp{k}")
            nc.sync.dma_start(out=wpt, in_=wp_ext[k * 128:(k + 1) * 128, :])
            wp_sb.append(wpt)

        # persistent per-chunk results
        vdw = [sb.tile([128, HW], bf16, name=f"vdw{g}", tag=f"vdw{g}")
               for g in range(G)]
        A_sb = [sb.tile([128, C], bf16, name=f"A{g}", tag=f"A{g}")
                for g in range(G)]
        qT = {}
        kT = {}
        ts_scale = [sb.tile([128, 1], f32, name=f"tss{g}", tag=f"tss{g}")
                    for g in range(G)]
        # single DVE scratch for TS+TT tap pairs (DVE-serial -> bufs=1)
        tmp = sb.tile([128, HW], bf16, name="dvtmp", tag="dvtmp")
        tmp3 = tmp.rearrange("p (h w) -> p h w", w=WW)


        # ---- pw conv + depthwise for one (tensor, chunk) ----------------
        def pw_dw(t, g, acc):
            """1x1 conv chunk g of tensor t into a zero-bordered [66,68]
            padded SBUF tile, then 3x3 depthwise into acc [128, HW] bf16
            (PE center-column taps + DVE TS/TT for the rest)."""
            pad = sb.tile([128, PY, PR], bf16, name=f"pad_{t}{g}", tag="pwpad",
                          bufs=4)
            # zero borders: top/bottom rows, left/right cols (gpsimd; idle)
            nc.gpsimd.memset(pad[:, 0:1, :], 0.0)
            nc.gpsimd.memset(pad[:, PY - 1:PY, :], 0.0)
            nc.gpsimd.memset(pad[:, 1:PY - 1, 0:1], 0.0)
            nc.gpsimd.memset(pad[:, 1:PY - 1, PR - 1:PR], 0.0)
            # 1x1 conv: bank-pair loop; each stationary w block serves 2 MMs
            for nb2 in range(NB // 2):
                pp0 = ps.tile([128, NBK], f32, name=f"pwa_{t}{g}{nb2}",
                              tag="ps_pw", bufs=3)
                pp1 = ps.tile([128, NBK], f32, name=f"pwb_{t}{g}{nb2}",
                              tag="ps_pw", bufs=3)
                for k in range(G):
                    lhs = w_sb[t][k][:, g * 128:(g + 1) * 128]
                    for j, pp in enumerate((pp0, pp1)):
                        nb = nb2 * 2 + j
                        nc.tensor.matmul(
                            pp, lhsT=lhs,
                            rhs=x_sb[k][:, nb * NBK:(nb + 1) * NBK],
                            start=(k == 0), stop=(k == G - 1),
                        )
                for j, pp in enumerate((pp0, pp1)):
                    nb = nb2 * 2 + j
                    dst = pad[:, 1 + nb * 8:1 + (nb + 1) * 8, 1:WW + 1]
                    src = pp.rearrange("p (h w) -> p h w", w=WW)
                    if (t, g) in PW_COPY_DVE:
                        nc.vector.tensor_copy(dst, src)
                    else:
                        nc.scalar.copy(dst, src)

            mode = _mode(t, g)
            pe_taps = TAPORD[:N_PE[mode]]
            dwc = dw_sb[t][g]
            acc3 = acc.rearrange("p (h w) -> p h w", w=WW)

            if pe_taps:
                # PE partial: accumulating diag-matmuls per bank pair
                dgt = dg_sb[(t, g)]
                for nb2 in range(NB // 2):
                    dp0 = ps.tile([128, NBK], f32, name=f"dwa_{t}{g}{nb2}",
                                  tag="ps_dw", bufs=3)
                    dp1 = ps.tile([128, NBK], f32, name=f"dwb_{t}{g}{nb2}",
                                  tag="ps_dw", bufs=3)
                    for ti, tap in enumerate(pe_taps):
                        dy, dx = tap // 3, tap % 3
                        lhs = dgt[:, ti * 128:(ti + 1) * 128]
                        for j, dp in enumerate((dp0, dp1)):
                            r0 = (nb2 * 2 + j) * 8
                            nc.tensor.matmul(
                                dp, lhsT=lhs,
                                rhs=pad[:, r0 + dy:r0 + dy + 8, dx:dx + WW],
                                start=(ti == 0), stop=(ti == len(pe_taps) - 1),
                            )
                    for j, dp in enumerate((dp0, dp1)):
                        nb = nb2 * 2 + j
                        if (t, g) in DW_COPY_DVE:
                            nc.vector.tensor_copy(
                                acc[:, nb * NBK:(nb + 1) * NBK], dp)
                        else:
                            nc.scalar.copy(acc[:, nb * NBK:(nb + 1) * NBK], dp)
                dve_taps = (3, 5, 0, 2, 6, 8) if mode == "col" else ()
            else:
                # all-DVE: center tap initializes acc, 8 TS+TT pairs follow
                nc.vector.tensor_scalar(
                    out=acc3, in0=pad[:, 1:1 + HH, 1:1 + WW],
                    scalar1=dwc[:, 4:5], scalar2=None, op0=AO.mult)
                dve_taps = (1, 7, 3, 5, 0, 2, 6, 8)

            # DVE taps: tmp = window * w[tap] (TS 4x), acc += tmp (TT 2x)
            gp = GP_TAPS.get((t, g), ())
            for tap in dve_taps:
                if tap in gp:
                    continue
                dy, dx = tap // 3, tap % 3
                nc.vector.tensor_scalar(
                    out=tmp3, in0=pad[:, dy:dy + HH, dx:dx + WW],
                    scalar1=dwc[:, tap:tap + 1], scalar2=None, op0=AO.mult)
                nc.vector.tensor_tensor(out=acc, in0=acc, in1=tmp,
                                        op=AO.add)
            # GPSIMD taps: in-place FMA on the otherwise-idle engine
            for tap in gp:
                dy, dx = tap // 3, tap % 3
                nc.gpsimd.scalar_tensor_tensor(
                    out=acc3, in0=pad[:, dy:dy + HH, dx:dx + WW],
                    scalar=dwc[:, tap:tap + 1], in1=acc3,
                    op0=AO.mult, op1=AO.add)
            return pad

        # ---- rsqrt of sum-of-squares along free dim ---------------------
        def rnorm(src, g, t, pad):
            """returns [128,1] f32 tile = 1/max(||src row||, eps).
            Squares scratch overwrites the chunk's dead pad tile."""
            padf = pad.rearrange("p h w -> p (h w)")
            ss = sb.tile([128, 1], f32, name=f"ss_{t}{g}", tag="nrm_ss",
                         bufs=2)
            nc.scalar.activation(out=padf[:, 0:HW], in_=src, func=AF.Square,
                                 accum_out=ss)
            nc.vector.tensor_scalar(out=ss, in0=ss, scalar1=EPS * EPS,
                                    scalar2=None, op0=AO.max)
            sr = sb.tile([128, 1], f32, name=f"sr_{t}{g}", tag="nrm_sr",
                         bufs=2)
            nc.scalar.activation(out=sr, in_=ss, func=AF.Sqrt)
            r0_ = sb.tile([128, 1], f32, name=f"r0_{t}{g}", tag="nrm_r0",
                          bufs=2)
            nc.vector.reciprocal(r0_, sr)
            # one Newton step: r = r0*(1.5 - 0.5*ss*r0^2)  (ACT sqrt is loose)
            tn = sb.tile([128, 1], f32, name=f"tn_{t}{g}", tag="nrm_tn",
                         bufs=2)
            nc.vector.tensor_tensor(out=tn, in0=r0_, in1=r0_, op=AO.mult)
            nc.vector.tensor_tensor(out=tn, in0=tn, in1=ss, op=AO.mult)
            nc.vector.tensor_scalar(out=tn, in0=tn, scalar1=-0.5, scalar2=1.5,
                                    op0=AO.mult, op1=AO.add)
            rinv = sb.tile([128, 1], f32, name=f"ri_{t}{g}", tag=f"ri_{t}{g}")
            nc.vector.tensor_tensor(out=rinv, in0=r0_, in1=tn, op=AO.mult)
            return rinv

        def post_qk(t, g, acc, pad):
            rinv = rnorm(acc, g, t, pad)
            if t == "q":
                # fold 1/|q| into the softmax exp scale (ACT: short queue)
                nc.scalar.activation(out=ts_scale[g], in_=tsc_sb[g],
                                     func=AF.Copy, scale=rinv)
            elif _mode(t, g) == "all":
                # tail chunk: normalize on ACT so the transpose doesn't
                # wait on the DVE tap backlog
                nc.scalar.activation(out=acc, in_=acc, func=AF.Copy,
                                     scale=rinv)
            else:
                # k-hat = k / |k| in place (DVE 4x), then transpose
                nc.vector.tensor_scalar(out=acc, in0=acc, scalar1=rinv,
                                        scalar2=None, op0=AO.mult)
            dstT = sb.tile([128, HW], bf16, name=f"{t}T{g}",
                           tag=f"{t}T", bufs=2)
            (qT if t == "q" else kT)[g] = dstT
            dst3 = dstT.rearrange("p (a c) -> p a c", c=128)
            if t == "k" and K_TRANSPOSE_ON_ACT:
                nc.scalar.dma_start(out=dst3, in_=acc, transpose=True)
            else:
                nc.sync.dma_start(out=dst3, in_=acc, transpose=True)

        # ---- attention + proj-weight fusion for one head-pair -----------
        def attn_pair(g):
            ap_ = ps.tile([128, 128], f32, name=f"attn{g}", tag="ps_attn",
                          bufs=1)
            for nck in range(32):
                nc.tensor.matmul(
                    ap_,
                    lhsT=qT[g][:, nck * 128:(nck + 1) * 128],
                    rhs=kT[g][:, nck * 128:(nck + 1) * 128],
                    start=(nck == 0), stop=(nck == 31))
            aexp = sb.tile([128, 128], bf16, name=f"aexp{g}", tag="aexp",
                           bufs=2)
            nc.gpsimd.memset(aexp, 0.0)
            sume = sb.tile([128, 1], f32, name=f"sume{g}", tag="sume", bufs=2)
            for blk in (0, 64):
                nc.scalar.activation(
                    out=aexp[blk:blk + 64, blk:blk + 64],
                    in_=ap_[blk:blk + 64, blk:blk + 64],
                    func=AF.Exp, scale=ts_scale[g][blk:blk + 64, :],
                    accum_out=sume[blk:blk + 64, :])
            rs = sb.tile([128, 1], f32, name=f"rs{g}", tag="rsum", bufs=2)
            nc.vector.reciprocal(rs, sume)
            # row-normalized attn (bf16); ACT keeps this off DVE's queue
            aexpS = sb.tile([128, 128], bf16, name=f"aexpS{g}", tag="aexpS",
                            bufs=2)
            nc.scalar.activation(out=aexpS, in_=aexp, func=AF.Copy, scale=rs)
            # A_g = attn_g.T @ wp_g  -> [128(e), 512(m)]
            pA = ps.tile([128, C], f32, name=f"pA{g}", tag="ps_A", bufs=1)
            nc.tensor.matmul(pA, lhsT=aexpS, rhs=wp_sb[g][:, :],
                             start=True, stop=True)
            nc.scalar.copy(A_sb[g], pA)

        # ======= main schedule ===========================================
        ORDER = ORDER_CFG
        for t, g in ORDER:
            if t == "A":
                attn_pair(g)
            elif t == "v":
                pw_dw("v", g, vdw[g])
            else:
                acc = sb.tile([128, HW], bf16, name=f"acc_{t}{g}", tag="acc",
                              bufs=3)
                pad = pw_dw(t, g, acc)
                post_qk(t, g, acc, pad)

        # ======= projection: y_m = sum_g A_g[:,m].T @ vdw_g ==============
        # paired banks -> one [128,1024] store; copies alternate ACT/DVE
        # (DVE is idle during proj) so the PE matmuls set the pace
        for m in range(G):
            for nb2 in range(NB // 2):
                yps = [ps.tile([128, NBK], f32, name=f"yp{m}{nb2}{j}",
                               tag="ps_dw", bufs=3) for j in (0, 1)]
                for g in range(G):
                    lhs = A_sb[g][:, m * 128:(m + 1) * 128]
                    for j, yp in enumerate(yps):
                        nb = nb2 * 2 + j
                        nc.tensor.matmul(
                            yp, lhsT=lhs,
                            rhs=vdw[g][:, nb * NBK:(nb + 1) * NBK],
                            start=(g == 0), stop=(g == G - 1))
                yt = sb.tile([128, 2 * NBK], f32, name=f"yt{m}{nb2}",
                             tag="ysb", bufs=2)
                nc.scalar.copy(yt[:, 0:NBK], yps[0])
                nc.vector.tensor_copy(yt[:, NBK:2 * NBK], yps[1])
                nc.sync.dma_start(
                    out=out_ext[m * 128:(m + 1) * 128,
                                nb2 * 2 * NBK:(nb2 + 1) * 2 * NBK],
                    in_=yt)

    nc.compile()
    return nc


def _prep_inputs(x, w_q, w_k, w_v, dw_q, dw_k, dw_v, w_proj, temperature):
    bf16 = ml_dtypes.bfloat16
    xb = np.ascontiguousarray(np.asarray(x, np.float32)).reshape(B, C, HW)
    dwf = {
        "q": np.asarray(dw_q, np.float32).reshape(C, 9),
        "k": np.asarray(dw_k, np.float32).reshape(C, 9),
        "v": np.asarray(dw_v, np.float32).reshape(C, 9),
    }
    # diag(dw tap) blocks for the PE path, packed [128, G*9*128], TAPORD order
    dg = {}
    idx = np.arange(128)
    for t in "qkv":
        arr = np.zeros((128, G * 9 * 128), np.float32)
        for g in range(G):
            for ti, tap in enumerate(TAPORD):
                arr[idx, (g * 9 + ti) * 128 + idx] = dwf[t][g * 128 + idx, tap]
        dg[t] = arr.astype(bf16)
    base = {
        "wq": np.ascontiguousarray(np.asarray(w_q, np.float32).T).astype(bf16),
        "wk": np.ascontiguousarray(np.asarray(w_k, np.float32).T).astype(bf16),
        "wv": np.ascontiguousarray(np.asarray(w_v, np.float32).T).astype(bf16),
        "wp": np.ascontiguousarray(np.asarray(w_proj, np.float32).T).astype(bf16),
        "dgq": dg["q"], "dgk": dg["k"], "dgv": dg["v"],
        "dwq": dwf["q"].copy(), "dwk": dwf["k"].copy(), "dwv": dwf["v"].copy(),
        "tsc": np.repeat(np.asarray(temperature, np.float32).reshape(HEADS),
                         D).reshape(C, 1).copy(),
    }
    in_maps = []
    for b in range(B):
        m = dict(base)
        m["x"] = xb[b].astype(bf16)
        in_maps.append(m)
    return in_maps


def run(trace=False, **inputs):
    from concourse.bass_utils import run_bass_kernel_spmd

    if "nc" not in _CACHE:
        _CACHE["nc"] = _build()
    nc = _CACHE["nc"]
    in_maps = _prep_inputs(**inputs)
    res = run_bass_kernel_spmd(nc, in_maps, core_ids=list(range(B)),
                               trace=trace)
    out = np.stack([np.asarray(res.results[b]["out"], np.float32)
                    for b in range(B)])
    return out.reshape(B, C, HH, WW), res


def kernel(**inputs):
    out, _ = run(trace=False, **inputs)
    return out


# revision 32
# speedup vs baseline: 1.0326x; 1.0326x over previous
"""Trainium2 Bass kernel for channel attention (XCA-style) over 8 NeuronCores.

Module: q/k/v = depthwise3x3(1x1conv(x)); L2-normalize q,k over spatial;
channel attention per head (softmax over d=64 channel dim); 1x1 proj.

Sharding: data-parallel over batch B=8 -> 1 batch item per core, no
collectives.  All shapes hardcoded; weights pre-transposed on host.

v2 design (per core, per 128-channel chunk):
- 1x1 convs: bf16 PE matmuls into PSUM, ACT-copied into a zero-bordered
  [66, 68] padded SBUF tile (stride 68 keeps window reads 4B-aligned).
- depthwise 3x3 split by tap across engines: PE takes the dx=1 column
  (3 taps) as accumulating diag-matmuls (diags precomputed on host),
  ACT drains the partial to SBUF, DVE adds the 6 dx={0,2} taps as
  TS(4x mode) + TT-add(2x mode) pairs.  Some chunks run all 9 taps on
  PE (DW_MODE knob) for engine balance.
- L2 norm: ACT Square+accum_out (scratch over the dead pad tile),
  Newton-refined rsqrt; q's 1/|q| folds into the softmax exp scale,
  k's into a DVE 4x row-scale.
- q/k transposed to [n, c] via DMA xbar transpose (free on compute).
- softmax 1/sum folds into a row-scale of exp(attn); attn@v and the
  output projection fuse: A_g = attn_g.T @ wp_g (tiny matmul), then
  y_m = sum_g A_g[:,m].T @ vdw_g -- the separate attn@v pass and its
  PSUM drain disappear.
"""

import os
import sys

import numpy as np

for _p in ("/opt/trn_rl_repo", "/root/.axon_site/_ro/trn_rl_repo"):
    if os.path.isdir(_p) and _p not in sys.path:
        sys.path.insert(0, _p)

import ml_dtypes

B, C, HH, WW = 8, 512, 64, 64
HEADS, D = 8, 64
HW = HH * WW          # 4096
G = C // 128          # 4 channel chunks of 128
NBK = 512             # matmul N (one PSUM bank of fp32)
NB = HW // NBK        # 8
PR = 66               # pad row stride (elems); data at [1+y, 1+x]
PY = HH + 2           # 66 pad rows
EPS = 1e-12

# PE tap order: center column (dx=1) first so start=True covers full banks
TAPORD = [4, 1, 7, 3, 5, 0, 2, 6, 8]
# per (t,g) depthwise split:
#   "col" = taps {4,1,7} on PE + 6 on DVE
#   "all" = all 9 taps on PE (q/k tail chunks: transpose gates on ACT,
#           not on DVE's tap backlog)
#   "dve" = all 9 taps on DVE (v chunks: vdw only needed at proj, so
#           their taps can trail off the critical path)
DW_MODE = {("q", 2): "all", ("k", 2): "all", ("q", 3): "all",
           ("k", 3): "all", ("v", 3): "all",
           ("v", 0): "dve", ("v", 1): "dve", ("v", 2): "dve"}
N_PE = {"col": 3, "all": 9, "dve": 0}
# chunks whose pw PSUM->pad copies run on DVE (fills DVE's startup idle)
PW_COPY_DVE = {("q", 0)}
# chunks whose dw-partial PSUM->acc copies run on DVE
DW_COPY_DVE = {("q", 0)}
# load x in quarters (vs halves) so the first pw bank-pair starts earlier
X_QUARTERS = True
# split q/k transposes into halves so attn matmuls start on the first
# half while the second still transfers
SPLIT_TRANSPOSE = False
# all-PE q/k chunks: square each drained dw PSUM bank in place (accum_out
# collects row-sums) so the norm is ready with the copies, taking the
# 3.7us Square off the transpose->attn critical path
SQ_FROM_PSUM = True
# (t,g) -> taps offloaded to GPSIMD STT (runs after that chunk's DVE taps)
GP_TAPS = {}
# route k transposes through the Activation HWDGE queue (parallel with
# q transposes on the SP queue) so attn pairs unblock ~5us earlier
K_TRANSPOSE_ON_ACT = False
# main schedule (program order drives Tile's priorities)
ORDER_CFG = [("q", 0), ("k", 0), ("q", 1), ("k", 1), ("v", 0), ("A", 0),
             ("q", 2), ("k", 2), ("v", 1), ("A", 1), ("v", 2), ("q", 3),
             ("k", 3), ("A", 2), ("v", 3), ("A", 3)]


def _mode(t, g):
    return DW_MODE.get((t, g), "col")


_CACHE = {}


def _build():
    """Build the single-core Bass program (SPMD: same program, per-core data)."""
    from contextlib import ExitStack

    import concourse.bass as bass
    import concourse.tile as tile
    from concourse import bacc, mybir

    f32 = mybir.dt.float32
    bf16 = mybir.dt.bfloat16
    AO = mybir.AluOpType
    AF = mybir.ActivationFunctionType

    nc = bacc.Bacc()

    x_ext = nc.declare_dram_parameter("x", [C, HW], bf16, isOutput=False)
    w_ext = {
        t: nc.declare_dram_parameter(f"w{t}", [C, C], bf16, isOutput=False)
        for t in "qkv"
    }
    wp_ext = nc.declare_dram_parameter("wp", [C, C], bf16, isOutput=False)
    # host-precomputed diag(dw tap) blocks, TAPORD order per chunk
    dg_ext = {
        t: nc.declare_dram_parameter(f"dg{t}", [128, G * 9 * 128], bf16,
                                     isOutput=False)
        for t in "qkv"
    }
    dw_ext = {
        t: nc.declare_dram_parameter(f"dw{t}", [C, 9], f32, isOutput=False)
        for t in "qkv"
    }
    tsc_ext = nc.declare_dram_parameter("tsc", [C, 1], f32, isOutput=False)
    out_ext = nc.declare_dram_parameter("out", [C, HW], f32, isOutput=True)

    with ExitStack() as ctx:
        tc = ctx.enter_context(tile.TileContext(nc))
        sb = ctx.enter_context(tc.tile_pool(name="sb", bufs=1))
        ps = ctx.enter_context(tc.tile_pool(name="ps", bufs=1, space="PSUM"))

        # ---- persistent loads -------------------------------------------
        x_sb = [None] * G
        w_sb = {t: [None] * G for t in "qkv"}
        # leading slices of x land early so pw bank-pair 0 starts ~4us in
        XCUT = HW // 4 if X_QUARTERS else HW // 2
        for k in range(G):
            wt = sb.tile([128, C], bf16, name=f"wq{k}", tag=f"wq{k}")
            nc.sync.dma_start(out=wt, in_=w_ext["q"][k * 128:(k + 1) * 128, :])
            w_sb["q"][k] = wt
            xg = sb.tile([128, HW], bf16, name=f"x{k}", tag=f"x{k}")
            nc.sync.dma_start(out=xg[:, 0:XCUT],
                              in_=x_ext[k * 128:(k + 1) * 128, 0:XCUT])
            x_sb[k] = xg
        if X_QUARTERS:
            for k in range(G):
                nc.sync.dma_start(out=x_sb[k][:, XCUT:HW // 2],
                                  in_=x_ext[k * 128:(k + 1) * 128,
                                            XCUT:HW // 2])
        for k in range(G):
            nc.sync.dma_start(out=x_sb[k][:, HW // 2:HW],
                              in_=x_ext[k * 128:(k + 1) * 128, HW // 2:HW])

        dg_sb = {}
        dw_sb = {t: [None] * G for t in "qkv"}
        tsc_sb = []

        def load_dg_dw(t):
            for g in range(G):
                n = N_PE[_mode(t, g)]
                if n:
                    d = sb.tile([128, n * 128], bf16, name=f"dg{t}{g}",
                                tag=f"dg{t}{g}")
                    nc.sync.dma_start(
                        out=d,
                        in_=dg_ext[t][:, g * 9 * 128:(g * 9 + n) * 128])
                    dg_sb[(t, g)] = d
                dd = sb.tile([128, 9], f32, name=f"dw{t}{g}", tag=f"dw{t}{g}")
                nc.sync.dma_start(out=dd,
                                  in_=dw_ext[t][g * 128:(g + 1) * 128, :])
                dw_sb[t][g] = dd

        load_dg_dw("q")
        for g in range(G):
            tg = sb.tile([128, 1], f32, name=f"tsc{g}", tag=f"tsc{g}")
            nc.sync.dma_start(out=tg, in_=tsc_ext[g * 128:(g + 1) * 128, :])
            tsc_sb.append(tg)
        for t in "kv":
            for k in range(G):
                wt = sb.tile([128, C], bf16, name=f"w{t}{k}", tag=f"w{t}{k}")
                nc.sync.dma_start(out=wt, in_=w_ext[t][k * 128:(k + 1) * 128, :])
                w_sb[t][k] = wt
            load_dg_dw(t)
        wp_sb = []
        for k in range(G):
            wpt = sb.tile([128, C], bf16, name=f"wp{k}", tag=f"wp{k}")
            nc.sync.dma_start(out=wpt, in_=wp_ext[k * 128:(k + 1) * 128, :])
            wp_sb.append(wpt)

        # persistent per-chunk results
        vdw = [sb.tile([128, HW], bf16, name=f"vdw{g}", tag=f"vdw{g}")
               for g in range(G)]
        A_sb = [sb.tile([128, C], bf16, name=f"A{g}", tag=f"A{g}")
                for g in range(G)]
        qT = {}
        kT = {}
        ts_scale = [sb.tile([128, 1], f32, name=f"tss{g}", tag=f"tss{g}")
                    for g in range(G)]
        # single DVE scratch for TS+TT tap pairs (DVE-serial -> bufs=1)
        tmp = sb.tile([128, HW], bf16, name="dvtmp", tag="dvtmp")
        tmp3 = tmp.rearrange("p (h w) -> p h w", w=WW)


        # ---- pw conv + depthwise for one (tensor, chunk) ----------------
        def pw_dw(t, g, acc):
            """1x1 conv chunk g of tensor t into a zero-bordered [66,68]
            padded SBUF tile, then 3x3 depthwise into acc [128, HW] bf16
            (PE center-column taps + DVE TS/TT for the rest)."""
            pad = sb.tile([128, PY, PR], bf16, name=f"pad_{t}{g}", tag="pwpad",
                          bufs=4)
            # zero borders: top/bottom rows, left/right cols (gpsimd; idle)
            nc.gpsimd.memset(pad[:, 0:1, :], 0.0)
            nc.gpsimd.memset(pad[:, PY - 1:PY, :], 0.0)
            nc.gpsimd.memset(pad[:, 1:PY - 1, 0:1], 0.0)
            nc.gpsimd.memset(pad[:, 1:PY - 1, PR - 1:PR], 0.0)
            # 1x1 conv: bank-pair loop; each stationary w block serves 2 MMs
            for nb2 in range(NB // 2):
                pp0 = ps.tile([128, NBK], f32, name=f"pwa_{t}{g}{nb2}",
                              tag="ps_pw", bufs=3)
                pp1 = ps.tile([128, NBK], f32, name=f"pwb_{t}{g}{nb2}",
                              tag="ps_pw", bufs=3)
                for k in range(G):
                    lhs = w_sb[t][k][:, g * 128:(g + 1) * 128]
                    for j, pp in enumerate((pp0, pp1)):
                        nb = nb2 * 2 + j
                        nc.tensor.matmul(
                            pp, lhsT=lhs,
                            rhs=x_sb[k][:, nb * NBK:(nb + 1) * NBK],
                            start=(k == 0), stop=(k == G - 1),
                        )
                for j, pp in enumerate((pp0, pp1)):
                    nb = nb2 * 2 + j
                    dst = pad[:, 1 + nb * 8:1 + (nb + 1) * 8, 1:WW + 1]
                    src = pp.rearrange("p (h w) -> p h w", w=WW)
                    if (t, g) in PW_COPY_DVE:
                        nc.vector.tensor_copy(dst, src)
                    else:
                        nc.scalar.copy(dst, src)

            mode = _mode(t, g)
            pe_taps = TAPORD[:N_PE[mode]]
            dwc = dw_sb[t][g]
            acc3 = acc.rearrange("p (h w) -> p h w", w=WW)

            ssp = None
            if N_PE[mode] == 9 and t != "v" and SQ_FROM_PSUM:
                ssp = sb.tile([128, NB], f32, name=f"ssp_{t}{g}", tag="ssp",
                              bufs=2)
                ss_part[(t, g)] = ssp
            if pe_taps:
                # PE partial: accumulating diag-matmuls per bank pair
                dgt = dg_sb[(t, g)]
                for nb2 in range(NB // 2):
                    dp0 = ps.tile([128, NBK], f32, name=f"dwa_{t}{g}{nb2}",
                                  tag="ps_dw", bufs=3)
                    dp1 = ps.tile([128, NBK], f32, name=f"dwb_{t}{g}{nb2}",
                                  tag="ps_dw", bufs=3)
                    for ti, tap in enumerate(pe_taps):
                        dy, dx = tap // 3, tap % 3
                        lhs = dgt[:, ti * 128:(ti + 1) * 128]
                        for j, dp in enumerate((dp0, dp1)):
                            r0 = (nb2 * 2 + j) * 8
                            nc.tensor.matmul(
                                dp, lhsT=lhs,
                                rhs=pad[:, r0 + dy:r0 + dy + 8, dx:dx + WW],
                                start=(ti == 0), stop=(ti == len(pe_taps) - 1),
                            )
                    for j, dp in enumerate((dp0, dp1)):
                        nb = nb2 * 2 + j
                        if (t, g) in DW_COPY_DVE:
                            nc.vector.tensor_copy(
                                acc[:, nb * NBK:(nb + 1) * NBK], dp)
                        else:
                            nc.scalar.copy(acc[:, nb * NBK:(nb + 1) * NBK], dp)
                        if ssp is not None:
                            nc.scalar.activation(out=dp, in_=dp,
                                                 func=AF.Square,
                                                 accum_out=ssp[:, nb:nb + 1])
                dve_taps = (3, 5, 0, 2, 6, 8) if mode == "col" else ()
            else:
                # all-DVE: center tap initializes acc, 8 TS+TT pairs follow
                nc.vector.tensor_scalar(
                    out=acc3, in0=pad[:, 1:1 + HH, 1:1 + WW],
                    scalar1=dwc[:, 4:5], scalar2=None, op0=AO.mult)
                dve_taps = (1, 7, 3, 5, 0, 2, 6, 8)

            # DVE taps: tmp = window * w[tap] (TS 4x), acc += tmp (TT 2x)
            gp = GP_TAPS.get((t, g), ())
            for tap in dve_taps:
                if tap in gp:
                    continue
                dy, dx = tap // 3, tap % 3
                nc.vector.tensor_scalar(
                    out=tmp3, in0=pad[:, dy:dy + HH, dx:dx + WW],
                    scalar1=dwc[:, tap:tap + 1], scalar2=None, op0=AO.mult)
                nc.vector.tensor_tensor(out=acc, in0=acc, in1=tmp,
                                        op=AO.add)
            # GPSIMD taps: in-place FMA on the otherwise-idle engine
            for tap in gp:
                dy, dx = tap // 3, tap % 3
                nc.gpsimd.scalar_tensor_tensor(
                    out=acc3, in0=pad[:, dy:dy + HH, dx:dx + WW],
                    scalar=dwc[:, tap:tap + 1], in1=acc3,
                    op0=AO.mult, op1=AO.add)
            return pad

        # ---- rsqrt of sum-of-squares along free dim ---------------------
        ss_part = {}

        def rnorm(src, g, t, pad):
            """returns [128,1] f32 tile = 1/max(||src row||, eps).
            Squares scratch overwrites the chunk's dead pad tile."""
            padf = pad.rearrange("p h w -> p (h w)")
            ss = sb.tile([128, 1], f32, name=f"ss_{t}{g}", tag="nrm_ss",
                         bufs=2)
            if (t, g) in ss_part:
                sc8 = sb.tile([128, NB], f32, name=f"sc8_{t}{g}", tag="sc8",
                              bufs=2)
                nc.scalar.activation(out=sc8, in_=ss_part[(t, g)],
                                     func=AF.Copy, accum_out=ss)
            else:
                nc.scalar.activation(out=padf[:, 0:HW], in_=src,
                                     func=AF.Square, accum_out=ss)
            nc.vector.tensor_scalar(out=ss, in0=ss, scalar1=EPS * EPS,
                                    scalar2=None, op0=AO.max)
            sr = sb.tile([128, 1], f32, name=f"sr_{t}{g}", tag="nrm_sr",
                         bufs=2)
            nc.scalar.activation(out=sr, in_=ss, func=AF.Sqrt)
            r0_ = sb.tile([128, 1], f32, name=f"r0_{t}{g}", tag="nrm_r0",
                          bufs=2)
            nc.vector.reciprocal(r0_, sr)
            # one Newton step: r = r0*(1.5 - 0.5*ss*r0^2)  (ACT sqrt is loose)
            tn = sb.tile([128, 1], f32, name=f"tn_{t}{g}", tag="nrm_tn",
                         bufs=2)
            nc.vector.tensor_tensor(out=tn, in0=r0_, in1=r0_, op=AO.mult)
            nc.vector.tensor_tensor(out=tn, in0=tn, in1=ss, op=AO.mult)
            nc.vector.tensor_scalar(out=tn, in0=tn, scalar1=-0.5, scalar2=1.5,
                                    op0=AO.mult, op1=AO.add)
            rinv = sb.tile([128, 1], f32, name=f"ri_{t}{g}", tag=f"ri_{t}{g}")
            nc.vector.tensor_tensor(out=rinv, in0=r0_, in1=tn, op=AO.mult)
            return rinv

        def post_qk(t, g, acc, pad):
            rinv = rnorm(acc, g, t, pad)
            if t == "q":
                # fold 1/|q| into the softmax exp scale (ACT: short queue)
                nc.scalar.activation(out=ts_scale[g], in_=tsc_sb[g],
                                     func=AF.Copy, scale=rinv)
            elif _mode(t, g) == "all":
                # tail chunk: normalize on ACT so the transpose doesn't
                # wait on the DVE tap backlog
                nc.scalar.activation(out=acc, in_=acc, func=AF.Copy,
                                     scale=rinv)
            else:
                # k-hat = k / |k| in place (DVE 4x), then transpose
                nc.vector.tensor_scalar(out=acc, in0=acc, scalar1=rinv,
                                        scalar2=None, op0=AO.mult)
            dstT = sb.tile([128, HW], bf16, name=f"{t}T{g}",
                           tag=f"{t}T", bufs=2)
            (qT if t == "q" else kT)[g] = dstT
            dst3 = dstT.rearrange("p (a c) -> p a c", c=128)
            if SPLIT_TRANSPOSE:
                nc.sync.dma_start(out=dst3[:, 0:16, :], in_=acc[:, 0:HW // 2],
                                  transpose=True)
                nc.sync.dma_start(out=dst3[:, 16:32, :], in_=acc[:, HW // 2:HW],
                                  transpose=True)
            else:
                nc.sync.dma_start(out=dst3, in_=acc, transpose=True)

        # ---- attention + proj-weight fusion for one head-pair -----------
        def attn_pair(g):
            ap_ = ps.tile([128, 128], f32, name=f"attn{g}", tag="ps_attn",
                          bufs=1)
            for nck in range(32):
                nc.tensor.matmul(
                    ap_,
                    lhsT=qT[g][:, nck * 128:(nck + 1) * 128],
                    rhs=kT[g][:, nck * 128:(nck + 1) * 128],
                    start=(nck == 0), stop=(nck == 31))
            aexp = sb.tile([128, 128], bf16, name=f"aexp{g}", tag="aexp",
                           bufs=2)
            nc.gpsimd.memset(aexp, 0.0)
            sume = sb.tile([128, 1], f32, name=f"sume{g}", tag="sume", bufs=2)
            for blk in (0, 64):
                nc.scalar.activation(
                    out=aexp[blk:blk + 64, blk:blk + 64],
                    in_=ap_[blk:blk + 64, blk:blk + 64],
                    func=AF.Exp, scale=ts_scale[g][blk:blk + 64, :],
                    accum_out=sume[blk:blk + 64, :])
            rs = sb.tile([128, 1], f32, name=f"rs{g}", tag="rsum", bufs=2)
            nc.vector.reciprocal(rs, sume)
            # row-normalized attn (bf16); ACT keeps this off DVE's queue
            aexpS = sb.tile([128, 128], bf16, name=f"aexpS{g}", tag="aexpS",
                            bufs=2)
            nc.scalar.activation(out=aexpS, in_=aexp, func=AF.Copy, scale=rs)
            # A_g = attn_g.T @ wp_g  -> [128(e), 512(m)]
            pA = ps.tile([128, C], f32, name=f"pA{g}", tag="ps_A", bufs=1)
            nc.tensor.matmul(pA, lhsT=aexpS, rhs=wp_sb[g][:, :],
                             start=True, stop=True)
            nc.scalar.copy(A_sb[g], pA)

        # ======= main schedule ===========================================
        ORDER = ORDER_CFG
        for t, g in ORDER:
            if t == "A":
                attn_pair(g)
            elif t == "v":
                pw_dw("v", g, vdw[g])
            else:
                acc = sb.tile([128, HW], bf16, name=f"acc_{t}{g}", tag="acc",
                              bufs=3)
                pad = pw_dw(t, g, acc)
                post_qk(t, g, acc, pad)

        # ======= projection: y_m = sum_g A_g[:,m].T @ vdw_g ==============
        # paired banks -> one [128,1024] store; copies alternate ACT/DVE
        # (DVE is idle during proj) so the PE matmuls set the pace
        for m in range(G):
            for nb2 in range(NB // 2):
                yps = [ps.tile([128, NBK], f32, name=f"yp{m}{nb2}{j}",
                               tag="ps_dw", bufs=3) for j in (0, 1)]
                for g in range(G):
                    lhs = A_sb[g][:, m * 128:(m + 1) * 128]
                    for j, yp in enumerate(yps):
                        nb = nb2 * 2 + j
                        nc.tensor.matmul(
                            yp, lhsT=lhs,
                            rhs=vdw[g][:, nb * NBK:(nb + 1) * NBK],
                            start=(g == 0), stop=(g == G - 1))
                yt = sb.tile([128, 2 * NBK], f32, name=f"yt{m}{nb2}",
                             tag="ysb", bufs=2)
                nc.scalar.copy(yt[:, 0:NBK], yps[0])
                nc.vector.tensor_copy(yt[:, NBK:2 * NBK], yps[1])
                nc.sync.dma_start(
                    out=out_ext[m * 128:(m + 1) * 128,
                                nb2 * 2 * NBK:(nb2 + 1) * 2 * NBK],
                    in_=yt)

    nc.compile()
    return nc


def _prep_inputs(x, w_q, w_k, w_v, dw_q, dw_k, dw_v, w_proj, temperature):
    bf16 = ml_dtypes.bfloat16
    xb = np.ascontiguousarray(np.asarray(x, np.float32)).reshape(B, C, HW)
    dwf = {
        "q": np.asarray(dw_q, np.float32).reshape(C, 9),
        "k": np.asarray(dw_k, np.float32).reshape(C, 9),
        "v": np.asarray(dw_v, np.float32).reshape(C, 9),
    }
    # diag(dw tap) blocks for the PE path, packed [128, G*9*128], TAPORD order
    dg = {}
    idx = np.arange(128)
    for t in "qkv":
        arr = np.zeros((128, G * 9 * 128), np.float32)
        for g in range(G):
            for ti, tap in enumerate(TAPORD):
                arr[idx, (g * 9 + ti) * 128 + idx] = dwf[t][g * 128 + idx, tap]
        dg[t] = arr.astype(bf16)
    base = {
        "wq": np.ascontiguousarray(np.asarray(w_q, np.float32).T).astype(bf16),
        "wk": np.ascontiguousarray(np.asarray(w_k, np.float32).T).astype(bf16),
        "wv": np.ascontiguousarray(np.asarray(w_v, np.float32).T).astype(bf16),
        "wp": np.ascontiguousarray(np.asarray(w_proj, np.float32).T).astype(bf16),
        "dgq": dg["q"], "dgk": dg["k"], "dgv": dg["v"],
        "dwq": dwf["q"].copy(), "dwk": dwf["k"].copy(), "dwv": dwf["v"].copy(),
        "tsc": np.repeat(np.asarray(temperature, np.float32).reshape(HEADS),
                         D).reshape(C, 1).copy(),
    }
    in_maps = []
    for b in range(B):
        m = dict(base)
        m["x"] = xb[b].astype(bf16)
        in_maps.append(m)
    return in_maps


def run(trace=False, **inputs):
    from concourse.bass_utils import run_bass_kernel_spmd

    if "nc" not in _CACHE:
        _CACHE["nc"] = _build()
    nc = _CACHE["nc"]
    in_maps = _prep_inputs(**inputs)
    res = run_bass_kernel_spmd(nc, in_maps, core_ids=list(range(B)),
                               trace=trace)
    out = np.stack([np.asarray(res.results[b]["out"], np.float32)
                    for b in range(B)])
    return out.reshape(B, C, HH, WW), res


def kernel(**inputs):
    out, _ = run(trace=False, **inputs)
    return out
